# revision 30
# baseline (speedup 1.0000x reference)
"""TRN2 Bass kernel for the NonLocal (full N^2 attention) block.

Contract: kernel(**inputs) takes the FULL inputs (x:[4,128,64,64] plus 4x
(W:[128,128], b:[128])) and returns the full [4,128,64,64] output.

Sharding: 8 cores = 4 batches x 2 query-halves (2048 query rows each).
Each core receives the full x[b] (keys/values span all 4096 positions) and
its query slice; outputs are disjoint [128,2048] slices -> no collectives.

Per-core pipeline (v2):
  phi   = Wph @ X + bph            [C, 4096]   (f32r conv, f16 storage)
  theta = A*(Wth @ Xq + bth)       [C, 2048]   (A = Schraudolph scale, f16)
  gT    = X^T @ Wg2^T              [4096, C]   Wg2 = 0.5*Wo@Wg  (Wo FOLDED:
                                               per-query normalization
                                               commutes through the channel
                                               conv, so the output conv is
                                               pre-applied to g and the wy
                                               matmul disappears)
  per 1024-wide q-group, streaming over 32 key-chunks of 128:
    scT  = phi_chunk^T @ theta_q   [128, 1024] (= A*score)
    E    = exp(scT/A)              ACT op (scale=1/A), OR on flagged chunks
           bitcast_bf16(int16(scT + BEXP))     (Schraudolph exp on the DVE,
                                               ~3% rel err, offloads the ACT)
    yT  += gT_chunk^T @ E          [C, 1024]   (PSUM accumulation)
    sums: bf16 pairwise tree on DVE; for the LAST q-group the late chunks
          bypass the tree and are absorbed directly into a persistent
          PSUM "rb" accumulator by ones-matmuls spread through the stream
          (shrinks the serial tail)
  rb    = allones^T @ sum_tiles    [C, 1024]   (partition reduce + broadcast)
  out   = (tanh(yT * recip(rb) + 0.5*bo_eff) + 1) * (Xq/2)

A ~12-matmul dummy burst on memset data runs at t=0 so the PE HAM clock
gate warms (~3.4us of busy) during the input DMA instead of 20us in.

All per-core inputs are packed into ONE [128, 4483] DRAM tensor so a single
dma_start loads everything behind one semaphore.
"""

import sys

for _p in ("/opt/trn_rl_repo",):
    if _p not in sys.path:
        sys.path.insert(0, _p)

import numpy as np

import concourse.bass as bass
import concourse.bacc as bacc
import concourse.mybir as mybir
import concourse.tile as tile
from concourse.bass_utils import run_bass_kernel_spmd

F32 = mybir.dt.float32
F32R = mybir.dt.float32r
F16 = mybir.dt.float16
BF16 = mybir.dt.bfloat16
I16 = mybir.dt.int16
AF = mybir.ActivationFunctionType

B, C, H, W = 4, 128, 64, 64
N = H * W            # 4096 key/value positions
NQ = N // 2          # 2048 query rows per core
QG = 512             # PSUM bank / max fp32 moving dim
GW = 2 * QG          # q-group width (1024)
NQG = NQ // GW       # 2 q-groups
MC = 32              # key chunks of 128
N_CORES = 8

SC_DT = F16
AV_DT = BF16

# Schraudolph fake-exp constants (bf16): bits(e^x) ~= int16(A*x + BEXP).
# A is folded into theta host-side; BEXP tuned for round-to-nearest convert.
A_EXP = 128.0 / np.log(2.0)          # 184.6650
BEXP_C0 = -5.6                       # centering offset, calibrated on host
BEXP = 16256.0 + BEXP_C0

# chunks whose exp runs on the DVE (Schraudolph) instead of the ACT engine.
# (qg, mc) pairs; keep them non-adjacent so each pairs with an ACT neighbor.
DVE_EXP = frozenset(
    [(0, mc) for mc in (22, 26, 30)]
    + [(1, mc) for mc in (13, 21, 28, 30)]
)

WARM_MMS = 16        # dummy FD=512 matmuls at t=0 to warm the PE clock gate
                     # AND bridge the DMA-gated conv phase so HAM never
                     # re-throttles before the stream saturates
DIRECT_RB = {0: 2, 1: 4}   # trailing chunks absorbed straight into rb psum
TREE_CAP = 3         # bf16 sum tree depth (tiles cover 2^cap chunks)
RB_CLAIM = {0: 22, 1: 6}   # chunk at which each qg claims its rb psum tiles

# packed input column offsets. Each core's x[b] is PERMUTED so its own
# query half comes first -> xq is a prefix of xf.
OFF_XF = 0
OFF_XQ = 0
OFF_WG = OFF_XF + N      # holds Wg2.T = (0.5*Wo@Wg).T
OFF_WTH = OFF_WG + C     # holds (A*Wth).T
OFF_WPH = OFF_WTH + C
OFF_BTH = OFF_WPH + C    # holds A*bth
OFF_BPH = OFF_BTH + 1
OFF_BO = OFF_BPH + 1     # holds 0.5*bo_eff (for the tanh-based sigmoid)
NW = OFF_BO + 1          # 4483

_CACHE = {}


def build_program():
    nc = bacc.Bacc("TRN2", target_bir_lowering=False, debug=False,
                   num_devices=N_CORES)

    inp = nc.declare_dram_parameter("inp", [C, NW], F32R, isOutput=False)
    out = nc.declare_dram_parameter("out", [C, NQ], F32, isOutput=True)

    with tile.TileContext(nc) as tc:
        with (
            tc.tile_pool(name="const", bufs=1) as const,
            tc.tile_pool(name="big", bufs=1) as big,
            tc.tile_pool(name="epool", bufs=6) as epool,
            tc.tile_pool(name="tpool", bufs=10) as tpool,
            tc.tile_pool(name="tailp", bufs=2) as tailp,
        ):
            inp_s = big.tile([C, NW], F32R)
            # weights/biases first, then xf in 4 ascending chunks
            nc.sync.dma_start(out=inp_s[:, OFF_WG:], in_=inp[:, OFF_WG:])
            for dk in range(4):
                sl = slice(dk * (N // 4), (dk + 1) * (N // 4))
                nc.sync.dma_start(out=inp_s[:, sl], in_=inp[:, sl])

            xf_s = inp_s[:, OFF_XF:OFF_XF + N]
            xq_s = inp_s[:, OFF_XQ:OFF_XQ + NQ]
            wg_s = inp_s[:, OFF_WG:OFF_WG + C]
            wth_s = inp_s[:, OFF_WTH:OFF_WTH + C]
            wph_s = inp_s[:, OFF_WPH:OFF_WPH + C]
            bth_s = inp_s[:, OFF_BTH:OFF_BTH + 1].bitcast(F32)
            bph_s = inp_s[:, OFF_BPH:OFF_BPH + 1].bitcast(F32)
            bo_s = inp_s[:, OFF_BO:OFF_BO + 1].bitcast(F32)

            # all-ones [C, C] used as lhsT for the partition-reduce+broadcast
            # matmuls and as src for the PE warm-up burst
            ones_sum = const.tile([C, C], AV_DT)
            nc.vector.memset(ones_sum, 1.0)
            warm_src = const.tile([C, QG], AV_DT)
            nc.vector.memset(warm_src, 0.0)

            # preload the exp ACT table set while input DMA streams
            warm = const.tile([1, 1], F32)
            nc.scalar.activation(out=warm, in_=ones_sum[0:1, 0:1], func=AF.Exp)

            th_s = big.tile([C, NQ], SC_DT)
            # 0.5*xq, for out = (tanh(...) + 1) * (xq/2)
            xqh_s = big.tile([C, NQ], F32)
            phi_s = big.tile([C, N], SC_DT)
            gT_s = big.tile([C, MC, C], AV_DT)

            # f16 copies of x and the conv weights: f32r matmuls run in the
            # slow fp32_mode=HIGH path (~4 cyc/row + serialized LDWEIGHTS),
            # f16 runs at 1 cyc/row with hidden weight loads. Casts run on
            # the idle ACT engine (Copy is in every table set).
            xf16 = big.tile([C, N], F16)
            w16 = const.tile([C, 3, C], F16)
            wg16 = w16[:, 0, :]
            wth16 = w16[:, 1, :]
            wph16 = w16[:, 2, :]
            for (dst, src) in ((wg16, wg_s), (wth16, wth_s), (wph16, wph_s)):
                with nc.allow_low_precision(reason="f16 conv weights"):
                    nc.scalar.activation(out=dst, in_=src.bitcast(F32),
                                         func=AF.Copy)
            cast_state = {"xf": 0}

            def ensure_cast(upto):
                # cast x DMA chunks to f16 lazily so the ACT queue doesn't
                # stall later ops behind casts of not-yet-arrived data
                while cast_state["xf"] <= min(upto, 3):
                    dk = cast_state["xf"]
                    cast_state["xf"] += 1
                    sl = slice(dk * (N // 4), (dk + 1) * (N // 4))
                    with nc.allow_low_precision(reason="f16 conv input"):
                        nc.scalar.activation(out=xf16[:, sl],
                                             in_=xf_s[:, sl].bitcast(F32),
                                             func=AF.Copy)

            ensure_cast(1)  # xq (query half) for the theta convs

            # ---- PSUM pools: sc 4 banks + yt 2 + mm 2 = 8 ----
            with (
                tc.tile_pool(name="ps_sc", bufs=2, space="PSUM") as ps_sc,
                tc.tile_pool(name="ps_y", bufs=1, space="PSUM") as ps_y,
                tc.tile_pool(name="ps_mm", bufs=2, space="PSUM") as ps_mm,
            ):
                # ---- PE warm-up burst: dummy matmuls on memset data keep the
                # PE busy ~4us so the HAM clock-gate opens during input DMA.
                # Writes land in the yt pool buffer; the first real AV matmul
                # (start=True) clears them. ----
                warm_ps = ps_y.tile([C, GW], F32, name="warm", tag="yt")
                for wi in range(WARM_MMS):
                    nc.tensor.matmul(warm_ps[:, :QG], lhsT=ones_sum,
                                     rhs=warm_src, start=True, stop=True)

                # ---- convs (bias adds on DVE). Only theta and the first
                # phi/gT tiles are emitted upfront; the rest interleave into
                # group 0's chunk loop. ----
                for j in range(NQ // QG):
                    ps = ps_mm.tile([C, QG], F32, tag="mm", name=f"cvt_{j}")
                    nc.tensor.matmul(
                        ps, lhsT=wth16, rhs=xf16[:, j * QG:(j + 1) * QG],
                        start=True, stop=True,
                    )
                    with nc.allow_low_precision(reason="theta storage dtype"):
                        nc.vector.tensor_scalar_add(
                            out=th_s[:, j * QG:(j + 1) * QG], in0=ps,
                            scalar1=bth_s)
                conv_state = {"phi": 0, "gt4": 0}

                def emit_phi_conv():
                    j = conv_state["phi"]
                    conv_state["phi"] += 1
                    ensure_cast(j // 2)
                    ps = ps_mm.tile([C, QG], F32, tag="mm", name=f"cvp_{j}")
                    nc.tensor.matmul(
                        ps, lhsT=wph16, rhs=xf16[:, j * QG:(j + 1) * QG],
                        start=True, stop=True,
                    )
                    with nc.allow_low_precision(reason="phi storage dtype"):
                        nc.vector.tensor_scalar_add(
                            out=phi_s[:, j * QG:(j + 1) * QG], in0=ps,
                            scalar1=bph_s)

                def emit_gt_conv4():
                    g4 = conv_state["gt4"]
                    conv_state["gt4"] += 1
                    ensure_cast(g4 // 2)
                    ps = ps_mm.tile([C, QG], F32, tag="mm", name=f"cvg_{g4}")
                    for k in range(4):
                        mc = 4 * g4 + k
                        nc.tensor.matmul(
                            ps[:, k * C:(k + 1) * C],
                            lhsT=xf16[:, mc * C:(mc + 1) * C], rhs=wg16,
                            start=True, stop=True,
                        )
                    with nc.allow_low_precision(reason="gT storage dtype"):
                        nc.vector.tensor_copy(
                            out=gT_s[:, 4 * g4:4 * g4 + 4, :], in_=ps)

                def ensure_convs(mc):
                    # stay a few chunks ahead of the attention stream; finish
                    # early so the mm psum pool frees up for the rb tiles
                    while conv_state["phi"] < min((mc + 12) // 4, N // QG):
                        emit_phi_conv()
                    while conv_state["gt4"] < min((mc + 8) // 4 + 2, MC // 4):
                        emit_gt_conv4()

                ensure_convs(-1)

                def emit_tail(st, nsplit=2):
                    # rb accumulators are already closed (last absorb had
                    # stop=True at mc 31) -> pure recip/mul/tanh/gate chain.
                    # nsplit=4 shortens the serial chain for the final tail.
                    qg = st["qg"]
                    q0 = qg * GW
                    w = GW // nsplit
                    yns = []
                    for h in range(nsplit):
                        sl = slice(h * w, (h + 1) * w)
                        rbi = tailp.tile([C, w], F32, name=f"rbi_{qg}_{h}",
                                         tag=f"rbi{nsplit}")
                        rb_src = st["rb"][h * 2 // nsplit]
                        if nsplit > 2:
                            off = (h % (nsplit // 2)) * w
                            rb_src = rb_src[:, off:off + w]
                        nc.vector.reciprocal_approx_fast(out=rbi, in_=rb_src)
                        yn = tailp.tile([C, w], F32, name=f"yn_{qg}_{h}",
                                        tag=f"yn{nsplit}")
                        nc.vector.tensor_mul(out=yn, in0=st["yt"][:, sl],
                                             in1=rbi)
                        yns.append(yn)
                        # sigmoid(wy+bo)*xq == (tanh((wy+bo)/2) + 1) * (xq/2);
                        # the 0.5 factor is folded into Wg2, bo_s holds
                        # 0.5*bo_eff. tanh shares the exp ACT table set.
                        tn = tailp.tile([C, w], F32, name=f"tn_{qg}_{h}",
                                        tag=f"tn{nsplit}")
                        nc.scalar.activation(
                            out=tn, in_=yn, func=AF.Tanh, bias=bo_s,
                            scale=1.0,
                        )
                        o = tailp.tile([C, w], F32, name=f"o_{qg}_{h}",
                                       tag=f"o{nsplit}")
                        nc.vector.scalar_tensor_tensor(
                            out=o, in0=tn, scalar=1.0,
                            in1=xqh_s[:, q0 + h * w:q0 + (h + 1) * w],
                            op0=mybir.AluOpType.add, op1=mybir.AluOpType.mult,
                        )
                        nc.sync.dma_start(
                            out=out[:, q0 + h * w:q0 + (h + 1) * w], in_=o)

                # ---- software-pipelined attention stream. Per (qg, mc) step
                # three stages; the sc matmuls of step i+1 are EMITTED before
                # the av matmuls of step i so the in-order PE queue never
                # stalls on the exp of step i. ----
                qstate = []
                for qg in range(NQG):
                    qstate.append({
                        "qg": qg,
                        "yt": ps_y.tile([C, GW], F32, name=f"yt_{qg}",
                                        tag="yt"),
                        "levels": [None] * TREE_CAP,
                        "rb": None,           # [h0, h1] psum accumulators
                        "rb_started": [False, False],
                        "rb_pending": [],     # sum tiles awaiting absorption
                        "n_rb": 0,
                        # tree tiles: full 2^cap groups + one leftover level
                        "total_rb": DIRECT_RB[qg]
                        + (MC - DIRECT_RB[qg]) // (1 << TREE_CAP)
                        + (1 if (MC - DIRECT_RB[qg]) % (1 << TREE_CAP) else 0),
                        "sc": {},
                        "et": {},
                    })

                def stage_sc(st, mc):
                    qg = st["qg"]
                    if qg == 0:
                        ensure_convs(mc)
                        if mc == 12:
                            # xqh is first needed by the qg0 tail; emitting it
                            # here keeps the DVE clear during startup
                            nc.vector.tensor_scalar_mul(
                                out=xqh_s, in0=xq_s.bitcast(F32), scalar1=0.5)
                    sc = ps_sc.tile([C, GW], F32, name=f"sc_{qg}_{mc}",
                                    tag="sc")
                    st["sc"][mc] = sc
                    q0 = qg * GW
                    for h in range(2):
                        nc.tensor.matmul(
                            sc[:, h * QG:(h + 1) * QG],
                            lhsT=phi_s[:, mc * C:(mc + 1) * C],
                            rhs=th_s[:, q0 + h * QG:q0 + (h + 1) * QG],
                            start=True, stop=True,
                        )

                def stage_exp(st, mc):
                    qg = st["qg"]
                    sc = st["sc"].pop(mc)
                    et = epool.tile([C, GW], AV_DT, name=f"et_{qg}_{mc}",
                                    tag="et")
                    st["et"][mc] = et
                    with nc.allow_low_precision(reason="exp output dtype"):
                        if (qg, mc) in DVE_EXP:
                            # Schraudolph: bits(e^x) = int16(A*x + BEXP);
                            # max(.,0) clamps scores < -88 to +0.0 (the
                            # int16 would go negative -> NaN bit pattern)
                            nc.vector.tensor_scalar(
                                out=et.bitcast(I16), in0=sc, scalar1=BEXP,
                                scalar2=0.0, op0=mybir.AluOpType.add,
                                op1=mybir.AluOpType.max)
                        else:
                            nc.scalar.activation(out=et, in_=sc, func=AF.Exp,
                                                 scale=1.0 / A_EXP)

                def rb_absorb(st, t, last):
                    for h in range(2):
                        nc.tensor.matmul(
                            st["rb"][h], lhsT=ones_sum,
                            rhs=t[:, h * QG:(h + 1) * QG],
                            start=not st["rb_started"][h], stop=last,
                        )
                        st["rb_started"][h] = True
                    st["n_rb"] += 1

                def stage_post(st, mc):
                    qg = st["qg"]
                    if qg == 1 and mc == 0:
                        emit_tail(qstate[0])
                    et = st["et"].pop(mc)
                    q0 = qg * GW
                    for h in range(2):
                        nc.tensor.matmul(
                            st["yt"][:, h * QG:(h + 1) * QG],
                            lhsT=gT_s[:, mc, :],
                            rhs=et[:, h * QG:(h + 1) * QG],
                            start=(mc == 0), stop=(mc == MC - 1),
                        )
                    if mc >= MC - DIRECT_RB[qg]:
                        st["rb_pending"].append(et)
                    else:
                        cur = et
                        lvl = 0
                        levels = st["levels"]
                        while lvl < TREE_CAP and levels[lvl] is not None:
                            t = tpool.tile([C, GW], AV_DT,
                                           name=f"tree_{qg}_{mc}_{lvl}",
                                           tag="tree")
                            with nc.allow_low_precision(reason="bf16 tree"):
                                nc.vector.tensor_add(out=t, in0=levels[lvl],
                                                     in1=cur)
                            levels[lvl] = None
                            cur = t
                            lvl += 1
                        if lvl == TREE_CAP:
                            st["rb_pending"].append(cur)
                        else:
                            levels[lvl] = cur
                        if mc == MC - DIRECT_RB[qg] - 1:
                            # flush leftover tree levels into one tile chain
                            rem = [t for t in levels if t is not None]
                            st["levels"] = [None] * TREE_CAP
                            while len(rem) > 1:
                                t = tpool.tile([C, GW], AV_DT,
                                               name=f"treefl_{qg}_{len(rem)}",
                                               tag="tree")
                                with nc.allow_low_precision(reason="bf16 tree"):
                                    nc.vector.tensor_add(out=t, in0=rem[0],
                                                         in1=rem[1])
                                rem = [t] + rem[2:]
                            st["rb_pending"].extend(rem)
                    if mc >= RB_CLAIM[qg]:
                        if st["rb"] is None:
                            st["rb"] = [
                                ps_mm.tile([C, QG], F32, tag="mm",
                                           name=f"rb_{qg}_0"),
                                ps_mm.tile([C, QG], F32, tag="mm",
                                           name=f"rb_{qg}_1"),
                            ]
                        while st["rb_pending"]:
                            t = st["rb_pending"].pop(0)
                            rb_absorb(st, t, st["n_rb"] == st["total_rb"] - 1)
                    if mc == MC - 1:
                        assert st["n_rb"] == st["total_rb"], (
                            st["n_rb"], st["total_rb"])

                # skew: sc leads exp by 1 step and av/absorb by 2, so the
                # in-order PE queue never waits on an exp that is <2 chunks
                # old, and ACT jitter doesn't stall the PE.
                steps = [(qg, mc) for qg in range(NQG) for mc in range(MC)]
                nsteps = len(steps)
                for i in range(nsteps + 2):
                    if i < nsteps:
                        stage_sc(qstate[steps[i][0]], steps[i][1])
                    if 1 <= i + 1 and 0 <= i - 1 < nsteps:
                        stage_exp(qstate[steps[i - 1][0]], steps[i - 1][1])
                    if 0 <= i - 2 < nsteps:
                        stage_post(qstate[steps[i - 2][0]], steps[i - 2][1])
                emit_tail(qstate[1], nsplit=4)

    nc.compile()
    return nc


def get_program():
    if "nc" not in _CACHE:
        _CACHE["nc"] = build_program()
    return _CACHE["nc"]


def make_in_maps(x, Wg, bg, Wth, bth, Wph, bph, Wo, bo):
    xr = np.ascontiguousarray(x.reshape(B, C, N), np.float32)
    bo_eff = (Wo.astype(np.float64) @ bg.astype(np.float64)
              + bo.astype(np.float64)).astype(np.float32)
    Wg2 = 0.5 * (Wo.astype(np.float64) @ Wg.astype(np.float64))
    wblock = np.concatenate([
        np.ascontiguousarray(Wg2.T, np.float32),
        np.ascontiguousarray(A_EXP * Wth.T, np.float32),
        np.ascontiguousarray(Wph.T, np.float32),
        (A_EXP * bth).reshape(C, 1).astype(np.float32),
        bph.reshape(C, 1).astype(np.float32),
        (0.5 * bo_eff).reshape(C, 1),
    ], axis=1)
    in_maps = []
    for core in range(N_CORES):
        b, qh = divmod(core, 2)
        mine = xr[b][:, qh * NQ:(qh + 1) * NQ]
        other = xr[b][:, (1 - qh) * NQ:(2 - qh) * NQ]
        packed = np.concatenate([mine, other, wblock], axis=1)
        in_maps.append({"inp": np.ascontiguousarray(packed)})
    return in_maps


def run(trace=False, **inputs):
    nc = get_program()
    in_maps = make_in_maps(**inputs)
    res = run_bass_kernel_spmd(nc, in_maps, core_ids=list(range(N_CORES)),
                               trace=trace)
    full = np.empty((B, C, N), np.float32)
    for core in range(N_CORES):
        b, qh = divmod(core, 2)
        full[b][:, qh * NQ:(qh + 1) * NQ] = res.results[core]["out"]
    return full.reshape(B, C, H, W), res


def kernel(**inputs) -> np.ndarray:
    out, _ = run(trace=False, **inputs)
    return out


# revision 31
# speedup vs baseline: 1.0291x; 1.0291x over previous
"""TRN2 Bass kernel for the NonLocal (full N^2 attention) block.

Contract: kernel(**inputs) takes the FULL inputs (x:[4,128,64,64] plus 4x
(W:[128,128], b:[128])) and returns the full [4,128,64,64] output.

Sharding: 8 cores = 4 batches x 2 query-halves (2048 query rows each).
Each core receives the full x[b] (keys/values span all 4096 positions) and
its query slice; outputs are disjoint [128,2048] slices -> no collectives.

Per-core pipeline (v2):
  phi   = Wph @ X + bph            [C, 4096]   (f32r conv, f16 storage)
  theta = A*(Wth @ Xq + bth)       [C, 2048]   (A = Schraudolph scale, f16)
  gT    = X^T @ Wg2^T              [4096, C]   Wg2 = 0.5*Wo@Wg  (Wo FOLDED:
                                               per-query normalization
                                               commutes through the channel
                                               conv, so the output conv is
                                               pre-applied to g and the wy
                                               matmul disappears)
  per 1024-wide q-group, streaming over 32 key-chunks of 128:
    scT  = phi_chunk^T @ theta_q   [128, 1024] (= A*score)
    E    = exp(scT/A)              ACT op (scale=1/A), OR on flagged chunks
           bitcast_bf16(int16(scT + BEXP))     (Schraudolph exp on the DVE,
                                               ~3% rel err, offloads the ACT)
    yT  += gT_chunk^T @ E          [C, 1024]   (PSUM accumulation)
    sums: bf16 pairwise tree on DVE; for the LAST q-group the late chunks
          bypass the tree and are absorbed directly into a persistent
          PSUM "rb" accumulator by ones-matmuls spread through the stream
          (shrinks the serial tail)
  rb    = allones^T @ sum_tiles    [C, 1024]   (partition reduce + broadcast)
  out   = (tanh(yT * recip(rb) + 0.5*bo_eff) + 1) * (Xq/2)

A ~12-matmul dummy burst on memset data runs at t=0 so the PE HAM clock
gate warms (~3.4us of busy) during the input DMA instead of 20us in.

All per-core inputs are packed into ONE [128, 4483] DRAM tensor so a single
dma_start loads everything behind one semaphore.
"""

import sys

for _p in ("/opt/trn_rl_repo",):
    if _p not in sys.path:
        sys.path.insert(0, _p)

import numpy as np

import concourse.bass as bass
import concourse.bacc as bacc
import concourse.mybir as mybir
import concourse.tile as tile
from concourse.bass_utils import run_bass_kernel_spmd

F32 = mybir.dt.float32
F32R = mybir.dt.float32r
F16 = mybir.dt.float16
BF16 = mybir.dt.bfloat16
I16 = mybir.dt.int16
AF = mybir.ActivationFunctionType

B, C, H, W = 4, 128, 64, 64
N = H * W            # 4096 key/value positions
NQ = N // 2          # 2048 query rows per core
QG = 512             # PSUM bank / max fp32 moving dim
GW = 2 * QG          # q-group width (1024)
NQG = NQ // GW       # 2 q-groups
MC = 32              # key chunks of 128
N_CORES = 8

SC_DT = F16
AV_DT = BF16

# Schraudolph fake-exp constants (bf16): bits(e^x) ~= int16(A*x + BEXP).
# A is folded into theta host-side; BEXP tuned for round-to-nearest convert.
A_EXP = 128.0 / np.log(2.0)          # 184.6650
BEXP_C0 = -5.6                       # centering offset, calibrated on host
BEXP = 16256.0 + BEXP_C0

# chunks whose exp runs on the DVE (Schraudolph) instead of the ACT engine.
# (qg, mc) pairs; keep them non-adjacent so each pairs with an ACT neighbor.
DVE_EXP = frozenset(
    [(0, mc) for mc in (22, 26, 30)]
    + [(1, mc) for mc in (13, 21, 28, 30)]
)

WARM_MMS = 16        # dummy FD=512 matmuls at t=0 to warm the PE clock gate
                     # AND bridge the DMA-gated conv phase so HAM never
                     # re-throttles before the stream saturates
DIRECT_RB = {0: 2, 1: 4}   # trailing chunks absorbed straight into rb psum
TREE_CAP = 3         # bf16 sum tree depth (tiles cover 2^cap chunks)
RB_CLAIM = {0: 22, 1: 6}   # chunk at which each qg claims its rb psum tiles

# packed input column offsets. Each core's x[b] is PERMUTED so its own
# query half comes first -> xq is a prefix of xf.
OFF_XF = 0
OFF_XQ = 0
OFF_WG = OFF_XF + N      # holds Wg2.T = (0.5*Wo@Wg).T
OFF_WTH = OFF_WG + C     # holds (A*Wth).T
OFF_WPH = OFF_WTH + C
OFF_BTH = OFF_WPH + C    # holds A*bth
OFF_BPH = OFF_BTH + 1
OFF_BO = OFF_BPH + 1     # holds 0.5*bo_eff (for the tanh-based sigmoid)
NW = OFF_BO + 1          # 4483

_CACHE = {}


def build_program():
    nc = bacc.Bacc("TRN2", target_bir_lowering=False, debug=False,
                   num_devices=N_CORES)

    inp = nc.declare_dram_parameter("inp", [C, NW], F32R, isOutput=False)
    out = nc.declare_dram_parameter("out", [C, NQ], F32, isOutput=True)

    with tile.TileContext(nc) as tc:
        with (
            tc.tile_pool(name="const", bufs=1) as const,
            tc.tile_pool(name="big", bufs=1) as big,
            tc.tile_pool(name="epool", bufs=6) as epool,
            tc.tile_pool(name="tpool", bufs=10) as tpool,
            tc.tile_pool(name="tailp", bufs=2) as tailp,
        ):
            inp_s = big.tile([C, NW], F32R)
            # weights/biases first, then xf in 4 ascending chunks
            nc.sync.dma_start(out=inp_s[:, OFF_WG:], in_=inp[:, OFF_WG:])
            for dk in range(4):
                sl = slice(dk * (N // 4), (dk + 1) * (N // 4))
                nc.sync.dma_start(out=inp_s[:, sl], in_=inp[:, sl])

            xf_s = inp_s[:, OFF_XF:OFF_XF + N]
            xq_s = inp_s[:, OFF_XQ:OFF_XQ + NQ]
            wg_s = inp_s[:, OFF_WG:OFF_WG + C]
            wth_s = inp_s[:, OFF_WTH:OFF_WTH + C]
            wph_s = inp_s[:, OFF_WPH:OFF_WPH + C]
            bth_s = inp_s[:, OFF_BTH:OFF_BTH + 1].bitcast(F32)
            bph_s = inp_s[:, OFF_BPH:OFF_BPH + 1].bitcast(F32)
            bo_s = inp_s[:, OFF_BO:OFF_BO + 1].bitcast(F32)

            # all-ones [C, C] used as lhsT for the partition-reduce+broadcast
            # matmuls and as src for the PE warm-up burst
            ones_sum = const.tile([C, C], AV_DT)
            nc.vector.memset(ones_sum, 1.0)
            warm_src = const.tile([C, QG], AV_DT)
            nc.vector.memset(warm_src, 0.0)

            # preload the exp ACT table set while input DMA streams
            warm = const.tile([1, 1], F32)
            nc.scalar.activation(out=warm, in_=ones_sum[0:1, 0:1], func=AF.Exp)

            th_s = big.tile([C, NQ], SC_DT)
            # 0.5*xq, for out = (tanh(...) + 1) * (xq/2)
            xqh_s = big.tile([C, NQ], F32)
            phi_s = big.tile([C, N], SC_DT)
            gT_s = big.tile([C, MC, C], AV_DT)

            # f16 copies of x and the conv weights: f32r matmuls run in the
            # slow fp32_mode=HIGH path (~4 cyc/row + serialized LDWEIGHTS),
            # f16 runs at 1 cyc/row with hidden weight loads. Casts run on
            # the idle ACT engine (Copy is in every table set).
            xf16 = big.tile([C, N], F16)
            w16 = const.tile([C, 3, C], F16)
            wg16 = w16[:, 0, :]
            wth16 = w16[:, 1, :]
            wph16 = w16[:, 2, :]
            for (dst, src) in ((wg16, wg_s), (wth16, wth_s), (wph16, wph_s)):
                with nc.allow_low_precision(reason="f16 conv weights"):
                    nc.scalar.activation(out=dst, in_=src.bitcast(F32),
                                         func=AF.Copy)
            cast_state = {"xf": 0}

            def ensure_cast(upto):
                # cast x DMA chunks to f16 lazily so the ACT queue doesn't
                # stall later ops behind casts of not-yet-arrived data
                while cast_state["xf"] <= min(upto, 3):
                    dk = cast_state["xf"]
                    cast_state["xf"] += 1
                    sl = slice(dk * (N // 4), (dk + 1) * (N // 4))
                    with nc.allow_low_precision(reason="f16 conv input"):
                        nc.scalar.activation(out=xf16[:, sl],
                                             in_=xf_s[:, sl].bitcast(F32),
                                             func=AF.Copy)

            ensure_cast(1)  # xq (query half) for the theta convs

            # ---- PSUM pools: sc 4 banks + yt 2 + mm 2 = 8 ----
            with (
                tc.tile_pool(name="ps_sc", bufs=2, space="PSUM") as ps_sc,
                tc.tile_pool(name="ps_y", bufs=1, space="PSUM") as ps_y,
                tc.tile_pool(name="ps_mm", bufs=2, space="PSUM") as ps_mm,
            ):
                # ---- PE warm-up burst: dummy matmuls on memset data keep the
                # PE busy ~4us so the HAM clock-gate opens during input DMA.
                # Writes land in the yt pool buffer; the first real AV matmul
                # (start=True) clears them. ----
                warm_ps = ps_y.tile([C, GW], F32, name="warm", tag="yt")
                for wi in range(WARM_MMS):
                    nc.tensor.matmul(warm_ps[:, :QG], lhsT=ones_sum,
                                     rhs=warm_src, start=True, stop=True)

                # ---- convs (bias adds on DVE). Only theta and the first
                # phi/gT tiles are emitted upfront; the rest interleave into
                # group 0's chunk loop. ----
                for j in range(NQ // QG):
                    ps = ps_mm.tile([C, QG], F32, tag="mm", name=f"cvt_{j}")
                    nc.tensor.matmul(
                        ps, lhsT=wth16, rhs=xf16[:, j * QG:(j + 1) * QG],
                        start=True, stop=True,
                    )
                    with nc.allow_low_precision(reason="theta storage dtype"):
                        nc.vector.tensor_scalar_add(
                            out=th_s[:, j * QG:(j + 1) * QG], in0=ps,
                            scalar1=bth_s)
                conv_state = {"phi": 0, "gt4": 0}

                def emit_phi_conv():
                    j = conv_state["phi"]
                    conv_state["phi"] += 1
                    ensure_cast(j // 2)
                    ps = ps_mm.tile([C, QG], F32, tag="mm", name=f"cvp_{j}")
                    nc.tensor.matmul(
                        ps, lhsT=wph16, rhs=xf16[:, j * QG:(j + 1) * QG],
                        start=True, stop=True,
                    )
                    with nc.allow_low_precision(reason="phi storage dtype"):
                        nc.vector.tensor_scalar_add(
                            out=phi_s[:, j * QG:(j + 1) * QG], in0=ps,
                            scalar1=bph_s)

                def emit_gt_conv4():
                    g4 = conv_state["gt4"]
                    conv_state["gt4"] += 1
                    ensure_cast(g4 // 2)
                    ps = ps_mm.tile([C, QG], F32, tag="mm", name=f"cvg_{g4}")
                    for k in range(4):
                        mc = 4 * g4 + k
                        nc.tensor.matmul(
                            ps[:, k * C:(k + 1) * C],
                            lhsT=xf16[:, mc * C:(mc + 1) * C], rhs=wg16,
                            start=True, stop=True,
                        )
                    with nc.allow_low_precision(reason="gT storage dtype"):
                        nc.vector.tensor_copy(
                            out=gT_s[:, 4 * g4:4 * g4 + 4, :], in_=ps)

                def ensure_convs(mc):
                    # stay a few chunks ahead of the attention stream; finish
                    # early so the mm psum pool frees up for the rb tiles
                    while conv_state["phi"] < min((mc + 12) // 4, N // QG):
                        emit_phi_conv()
                    while conv_state["gt4"] < min((mc + 8) // 4 + 2, MC // 4):
                        emit_gt_conv4()

                ensure_convs(-1)

                def emit_tail(st, nsplit=2):
                    # rb accumulators are already closed (last absorb had
                    # stop=True at mc 31) -> pure recip/mul/tanh/gate chain.
                    # nsplit=4 shortens the serial chain for the final tail.
                    qg = st["qg"]
                    q0 = qg * GW
                    w = GW // nsplit
                    yns = []
                    for h in range(nsplit):
                        sl = slice(h * w, (h + 1) * w)
                        rbi = tailp.tile([C, w], F32, name=f"rbi_{qg}_{h}",
                                         tag=f"rbi{nsplit}")
                        rb_src = st["rb"][h * 2 // nsplit]
                        if nsplit > 2:
                            off = (h % (nsplit // 2)) * w
                            rb_src = rb_src[:, off:off + w]
                        nc.vector.reciprocal_approx_fast(out=rbi, in_=rb_src)
                        yn = tailp.tile([C, w], F32, name=f"yn_{qg}_{h}",
                                        tag=f"yn{nsplit}")
                        nc.vector.tensor_mul(out=yn, in0=st["yt"][:, sl],
                                             in1=rbi)
                        yns.append(yn)
                        # sigmoid(wy+bo)*xq == (tanh((wy+bo)/2) + 1) * (xq/2);
                        # the 0.5 factor is folded into Wg2, bo_s holds
                        # 0.5*bo_eff. tanh shares the exp ACT table set.
                        tn = tailp.tile([C, w], F32, name=f"tn_{qg}_{h}",
                                        tag=f"tn{nsplit}")
                        nc.scalar.activation(
                            out=tn, in_=yn, func=AF.Tanh, bias=bo_s,
                            scale=1.0,
                        )
                        o = tailp.tile([C, w], F32, name=f"o_{qg}_{h}",
                                       tag=f"o{nsplit}")
                        nc.vector.scalar_tensor_tensor(
                            out=o, in0=tn, scalar=1.0,
                            in1=xqh_s[:, q0 + h * w:q0 + (h + 1) * w],
                            op0=mybir.AluOpType.add, op1=mybir.AluOpType.mult,
                        )
                        nc.sync.dma_start(
                            out=out[:, q0 + h * w:q0 + (h + 1) * w], in_=o)

                # ---- software-pipelined attention stream. Per (qg, mc) step
                # three stages; the sc matmuls of step i+1 are EMITTED before
                # the av matmuls of step i so the in-order PE queue never
                # stalls on the exp of step i. ----
                qstate = []
                for qg in range(NQG):
                    qstate.append({
                        "qg": qg,
                        "yt": ps_y.tile([C, GW], F32, name=f"yt_{qg}",
                                        tag="yt"),
                        "levels": [None] * TREE_CAP,
                        "rb": None,           # [h0, h1] psum accumulators
                        "rb_started": [False, False],
                        "rb_pending": [],     # sum tiles awaiting absorption
                        "n_rb": 0,
                        # tree tiles: full 2^cap groups + one leftover level
                        "total_rb": DIRECT_RB[qg]
                        + (MC - DIRECT_RB[qg]) // (1 << TREE_CAP)
                        + (1 if (MC - DIRECT_RB[qg]) % (1 << TREE_CAP) else 0),
                        "sc": {},
                        "et": {},
                    })

                def stage_sc(st, mc):
                    qg = st["qg"]
                    if qg == 0:
                        ensure_convs(mc)
                        if mc == 12:
                            # xqh is first needed by the qg0 tail; emitting it
                            # here keeps the DVE clear during startup
                            nc.vector.tensor_scalar_mul(
                                out=xqh_s, in0=xq_s.bitcast(F32), scalar1=0.5)
                    sc = ps_sc.tile([C, GW], F32, name=f"sc_{qg}_{mc}",
                                    tag="sc")
                    st["sc"][mc] = sc
                    q0 = qg * GW
                    for h in range(2):
                        nc.tensor.matmul(
                            sc[:, h * QG:(h + 1) * QG],
                            lhsT=phi_s[:, mc * C:(mc + 1) * C],
                            rhs=th_s[:, q0 + h * QG:q0 + (h + 1) * QG],
                            start=True, stop=True,
                        )

                def stage_exp(st, mc):
                    qg = st["qg"]
                    sc = st["sc"].pop(mc)
                    et = epool.tile([C, GW], AV_DT, name=f"et_{qg}_{mc}",
                                    tag="et")
                    st["et"][mc] = et
                    with nc.allow_low_precision(reason="exp output dtype"):
                        if (qg, mc) in DVE_EXP:
                            # Schraudolph: bits(e^x) = int16(A*x + BEXP);
                            # max(.,0) clamps scores < -88 to +0.0 (the
                            # int16 would go negative -> NaN bit pattern)
                            nc.vector.tensor_scalar(
                                out=et.bitcast(I16), in0=sc, scalar1=BEXP,
                                scalar2=0.0, op0=mybir.AluOpType.add,
                                op1=mybir.AluOpType.max)
                        else:
                            nc.scalar.activation(out=et, in_=sc, func=AF.Exp,
                                                 scale=1.0 / A_EXP)

                def rb_absorb(st, t, last):
                    for h in range(2):
                        nc.tensor.matmul(
                            st["rb"][h], lhsT=ones_sum,
                            rhs=t[:, h * QG:(h + 1) * QG],
                            start=not st["rb_started"][h], stop=last,
                        )
                        st["rb_started"][h] = True
                    st["n_rb"] += 1

                def stage_post(st, mc):
                    qg = st["qg"]
                    if qg == 1 and mc == 0:
                        emit_tail(qstate[0])
                    et = st["et"].pop(mc)
                    q0 = qg * GW
                    for h in range(2):
                        nc.tensor.matmul(
                            st["yt"][:, h * QG:(h + 1) * QG],
                            lhsT=gT_s[:, mc, :],
                            rhs=et[:, h * QG:(h + 1) * QG],
                            start=(mc == 0), stop=(mc == MC - 1),
                        )
                    if mc >= MC - DIRECT_RB[qg]:
                        st["rb_pending"].append(et)
                    else:
                        cur = et
                        lvl = 0
                        levels = st["levels"]
                        while lvl < TREE_CAP and levels[lvl] is not None:
                            t = tpool.tile([C, GW], AV_DT,
                                           name=f"tree_{qg}_{mc}_{lvl}",
                                           tag="tree")
                            with nc.allow_low_precision(reason="bf16 tree"):
                                nc.vector.tensor_add(out=t, in0=levels[lvl],
                                                     in1=cur)
                            levels[lvl] = None
                            cur = t
                            lvl += 1
                        if lvl == TREE_CAP:
                            st["rb_pending"].append(cur)
                        else:
                            levels[lvl] = cur
                        if mc == MC - DIRECT_RB[qg] - 1:
                            # flush leftover tree levels into one tile chain
                            rem = [t for t in levels if t is not None]
                            st["levels"] = [None] * TREE_CAP
                            while len(rem) > 1:
                                t = tpool.tile([C, GW], AV_DT,
                                               name=f"treefl_{qg}_{len(rem)}",
                                               tag="tree")
                                with nc.allow_low_precision(reason="bf16 tree"):
                                    nc.vector.tensor_add(out=t, in0=rem[0],
                                                         in1=rem[1])
                                rem = [t] + rem[2:]
                            st["rb_pending"].extend(rem)
                    if mc >= RB_CLAIM[qg]:
                        if st["rb"] is None:
                            st["rb"] = [
                                ps_mm.tile([C, QG], F32, tag="mm",
                                           name=f"rb_{qg}_0"),
                                ps_mm.tile([C, QG], F32, tag="mm",
                                           name=f"rb_{qg}_1"),
                            ]
                        while st["rb_pending"]:
                            t = st["rb_pending"].pop(0)
                            rb_absorb(st, t, st["n_rb"] == st["total_rb"] - 1)
                    if mc == MC - 1:
                        assert st["n_rb"] == st["total_rb"], (
                            st["n_rb"], st["total_rb"])

                # skew: sc leads exp by 1 step and av/absorb by 2, so the
                # in-order PE queue never waits on an exp that is <2 chunks
                # old, and ACT jitter doesn't stall the PE.
                steps = [(qg, mc) for qg in range(NQG) for mc in range(MC)]
                nsteps = len(steps)
                for i in range(nsteps + 2):
                    if i < nsteps:
                        stage_sc(qstate[steps[i][0]], steps[i][1])
                    if 1 <= i + 1 and 0 <= i - 1 < nsteps:
                        stage_exp(qstate[steps[i - 1][0]], steps[i - 1][1])
                    if 0 <= i - 2 < nsteps:
                        stage_post(qstate[steps[i - 2][0]], steps[i - 2][1])
                emit_tail(qstate[1])

    nc.compile()
    return nc


def get_program():
    if "nc" not in _CACHE:
        _CACHE["nc"] = build_program()
    return _CACHE["nc"]


def make_in_maps(x, Wg, bg, Wth, bth, Wph, bph, Wo, bo):
    xr = np.ascontiguousarray(x.reshape(B, C, N), np.float32)
    bo_eff = (Wo.astype(np.float64) @ bg.astype(np.float64)
              + bo.astype(np.float64)).astype(np.float32)
    Wg2 = 0.5 * (Wo.astype(np.float64) @ Wg.astype(np.float64))
    wblock = np.concatenate([
        np.ascontiguousarray(Wg2.T, np.float32),
        np.ascontiguousarray(A_EXP * Wth.T, np.float32),
        np.ascontiguousarray(Wph.T, np.float32),
        (A_EXP * bth).reshape(C, 1).astype(np.float32),
        bph.reshape(C, 1).astype(np.float32),
        (0.5 * bo_eff).reshape(C, 1),
    ], axis=1)
    in_maps = []
    for core in range(N_CORES):
        b, qh = divmod(core, 2)
        mine = xr[b][:, qh * NQ:(qh + 1) * NQ]
        other = xr[b][:, (1 - qh) * NQ:(2 - qh) * NQ]
        packed = np.concatenate([mine, other, wblock], axis=1)
        in_maps.append({"inp": np.ascontiguousarray(packed)})
    return in_maps


def run(trace=False, **inputs):
    nc = get_program()
    in_maps = make_in_maps(**inputs)
    res = run_bass_kernel_spmd(nc, in_maps, core_ids=list(range(N_CORES)),
                               trace=trace)
    full = np.empty((B, C, N), np.float32)
    for core in range(N_CORES):
        b, qh = divmod(core, 2)
        full[b][:, qh * NQ:(qh + 1) * NQ] = res.results[core]["out"]
    return full.reshape(B, C, H, W), res


def kernel(**inputs) -> np.ndarray:
    out, _ = run(trace=False, **inputs)
    return out


# revision 38
# speedup vs baseline: 1.0335x; 1.0042x over previous
"""TRN2 Bass kernel for the NonLocal (full N^2 attention) block.

Contract: kernel(**inputs) takes the FULL inputs (x:[4,128,64,64] plus 4x
(W:[128,128], b:[128])) and returns the full [4,128,64,64] output.

Sharding: 8 cores = 4 batches x 2 query-halves (2048 query rows each).
Each core receives the full x[b] (keys/values span all 4096 positions) and
its query slice; outputs are disjoint [128,2048] slices -> no collectives.

Per-core pipeline (v2):
  phi   = Wph @ X + bph            [C, 4096]   (f32r conv, f16 storage)
  theta = A*(Wth @ Xq + bth)       [C, 2048]   (A = Schraudolph scale, f16)
  gT    = X^T @ Wg2^T              [4096, C]   Wg2 = 0.5*Wo@Wg  (Wo FOLDED:
                                               per-query normalization
                                               commutes through the channel
                                               conv, so the output conv is
                                               pre-applied to g and the wy
                                               matmul disappears)
  per 1024-wide q-group, streaming over 32 key-chunks of 128:
    scT  = phi_chunk^T @ theta_q   [128, 1024] (= A*score)
    E    = exp(scT/A)              ACT op (scale=1/A), OR on flagged chunks
           bitcast_bf16(int16(scT + BEXP))     (Schraudolph exp on the DVE,
                                               ~3% rel err, offloads the ACT)
    yT  += gT_chunk^T @ E          [C, 1024]   (PSUM accumulation)
    sums: bf16 pairwise tree on DVE; for the LAST q-group the late chunks
          bypass the tree and are absorbed directly into a persistent
          PSUM "rb" accumulator by ones-matmuls spread through the stream
          (shrinks the serial tail)
  rb    = allones^T @ sum_tiles    [C, 1024]   (partition reduce + broadcast)
  out   = (tanh(yT * recip(rb) + 0.5*bo_eff) + 1) * (Xq/2)

A ~12-matmul dummy burst on memset data runs at t=0 so the PE HAM clock
gate warms (~3.4us of busy) during the input DMA instead of 20us in.

All per-core inputs are packed into ONE [128, 4483] DRAM tensor so a single
dma_start loads everything behind one semaphore.
"""

import sys

for _p in ("/opt/trn_rl_repo",):
    if _p not in sys.path:
        sys.path.insert(0, _p)

import numpy as np

import concourse.bass as bass
import concourse.bacc as bacc
import concourse.mybir as mybir
import concourse.tile as tile
from concourse.bass_utils import run_bass_kernel_spmd

F32 = mybir.dt.float32
F32R = mybir.dt.float32r
F16 = mybir.dt.float16
BF16 = mybir.dt.bfloat16
I16 = mybir.dt.int16
AF = mybir.ActivationFunctionType

B, C, H, W = 4, 128, 64, 64
N = H * W            # 4096 key/value positions
NQ = N // 2          # 2048 query rows per core
QG = 512             # PSUM bank / max fp32 moving dim
GW = 2 * QG          # q-group width (1024)
NQG = NQ // GW       # 2 q-groups
MC = 32              # key chunks of 128
N_CORES = 8

SC_DT = F16
AV_DT = BF16

# Schraudolph fake-exp constants (bf16): bits(e^x) ~= int16(A*x + BEXP).
# A is folded into theta host-side; BEXP tuned for round-to-nearest convert.
A_EXP = 128.0 / np.log(2.0)          # 184.6650
BEXP_C0 = -5.6                       # centering offset, calibrated on host
BEXP = 16256.0 + BEXP_C0

# chunks whose exp runs on the DVE (Schraudolph) instead of the ACT engine.
# (qg, mc) pairs; keep them non-adjacent so each pairs with an ACT neighbor.
DVE_EXP = frozenset(
    [(0, mc) for mc in (22, 26, 30)]
    + [(1, mc) for mc in (13, 21, 28, 30)]
)

WARM_MMS = 9         # dummy FD=512 matmuls at t=0 to warm the PE clock gate
                     # AND bridge the DMA-gated conv phase so HAM never
                     # re-throttles before the stream saturates
DIRECT_RB = {0: 2, 1: 4}   # trailing chunks absorbed straight into rb psum
TREE_CAP = 3         # bf16 sum tree depth (tiles cover 2^cap chunks)
RB_CLAIM = {0: 22, 1: 6}   # chunk at which each qg claims its rb psum tiles

# packed input column offsets, in f32 columns. x and the conv weights are
# packed as f16 PAIRS host-side (halves the DMA, kills the on-chip casts);
# each core's x[b] is PERMUTED so its own query half comes first.
OFF_X16 = 0              # [C, N/2] f32 cols = [C, N] f16 x
OFF_WG = OFF_X16 + N // 2    # f16 (0.5*Wo@Wg).T, 64 f32 cols
OFF_WTH = OFF_WG + C // 2    # f16 (A*Wth).T
OFF_WPH = OFF_WTH + C // 2   # f16 Wph.T
OFF_BTH = OFF_WPH + C // 2   # f32 A*bth
OFF_BPH = OFF_BTH + 1
OFF_BO = OFF_BPH + 1     # holds 0.5*bo_eff (for the tanh-based sigmoid)
NW = OFF_BO + 1          # 2243

_CACHE = {}


def build_program():
    nc = bacc.Bacc("TRN2", target_bir_lowering=False, debug=False,
                   num_devices=N_CORES)

    inp = nc.declare_dram_parameter("inp", [C, NW], F32, isOutput=False)
    out = nc.declare_dram_parameter("out", [C, NQ], F32, isOutput=True)

    with tile.TileContext(nc) as tc:
        with (
            tc.tile_pool(name="const", bufs=1) as const,
            tc.tile_pool(name="big", bufs=1) as big,
            tc.tile_pool(name="epool", bufs=6) as epool,
            tc.tile_pool(name="tpool", bufs=10) as tpool,
            tc.tile_pool(name="tailp", bufs=2) as tailp,
        ):
            inp_s = big.tile([C, NW], F32)
            # weights/biases first, then f16-x in 4 ascending key chunks
            nc.sync.dma_start(out=inp_s[:, OFF_WG:], in_=inp[:, OFF_WG:])
            for dk in range(4):
                sl = slice(dk * (N // 8), (dk + 1) * (N // 8))
                nc.sync.dma_start(out=inp_s[:, sl], in_=inp[:, sl])

            xf16 = inp_s[:, OFF_X16:OFF_X16 + N // 2].bitcast(F16)  # [C, N]
            xq16 = xf16[:, :NQ]
            wg16 = inp_s[:, OFF_WG:OFF_WG + C // 2].bitcast(F16)
            wth16 = inp_s[:, OFF_WTH:OFF_WTH + C // 2].bitcast(F16)
            wph16 = inp_s[:, OFF_WPH:OFF_WPH + C // 2].bitcast(F16)
            bth_s = inp_s[:, OFF_BTH:OFF_BTH + 1]
            bph_s = inp_s[:, OFF_BPH:OFF_BPH + 1]
            bo_s = inp_s[:, OFF_BO:OFF_BO + 1]

            # all-ones [C, C] used as lhsT for the partition-reduce+broadcast
            # matmuls and as src for the PE warm-up burst
            ones_sum = const.tile([C, C], AV_DT)
            nc.vector.memset(ones_sum, 1.0)
            warm_src = const.tile([C, QG], AV_DT)
            nc.vector.memset(warm_src, 0.0)

            # preload the exp ACT table set while input DMA streams
            warm = const.tile([1, 1], F32)
            nc.scalar.activation(out=warm, in_=ones_sum[0:1, 0:1], func=AF.Exp)

            th_s = big.tile([C, NQ], SC_DT)
            # 0.5*xq, for out = (tanh(...) + 1) * (xq/2)
            xqh_s = big.tile([C, NQ], F32)
            phi_s = big.tile([C, N], SC_DT)
            gT_s = big.tile([C, MC, C], AV_DT)

            # ---- PSUM pools: sc 4 banks + yt 2 + mm 2 = 8 ----
            with (
                tc.tile_pool(name="ps_sc", bufs=2, space="PSUM") as ps_sc,
                tc.tile_pool(name="ps_y", bufs=1, space="PSUM") as ps_y,
                tc.tile_pool(name="ps_mm", bufs=2, space="PSUM") as ps_mm,
            ):
                # ---- PE warm-up burst: dummy matmuls on memset data keep the
                # PE busy ~4us so the HAM clock-gate opens during input DMA.
                # Writes land in the yt pool buffer; the first real AV matmul
                # (start=True) clears them. ----
                warm_ps = ps_y.tile([C, GW], F32, name="warm", tag="yt")
                for wi in range(WARM_MMS):
                    nc.tensor.matmul(warm_ps[:, :QG], lhsT=ones_sum,
                                     rhs=warm_src, start=True, stop=True)

                # ---- convs (bias adds on DVE). Only theta and the first
                # phi/gT tiles are emitted upfront; the rest interleave into
                # group 0's chunk loop. ----
                for j in range(NQ // QG):
                    ps = ps_mm.tile([C, QG], F32, tag="mm", name=f"cvt_{j}")
                    nc.tensor.matmul(
                        ps, lhsT=wth16, rhs=xf16[:, j * QG:(j + 1) * QG],
                        start=True, stop=True,
                    )
                    with nc.allow_low_precision(reason="theta storage dtype"):
                        nc.vector.tensor_scalar_add(
                            out=th_s[:, j * QG:(j + 1) * QG], in0=ps,
                            scalar1=bth_s)
                conv_state = {"phi": 0, "gt4": 0}

                def emit_phi_conv():
                    j = conv_state["phi"]
                    conv_state["phi"] += 1
                    ps = ps_mm.tile([C, QG], F32, tag="mm", name=f"cvp_{j}")
                    nc.tensor.matmul(
                        ps, lhsT=wph16, rhs=xf16[:, j * QG:(j + 1) * QG],
                        start=True, stop=True,
                    )
                    with nc.allow_low_precision(reason="phi storage dtype"):
                        nc.vector.tensor_scalar_add(
                            out=phi_s[:, j * QG:(j + 1) * QG], in0=ps,
                            scalar1=bph_s)

                def emit_gt_conv4():
                    g4 = conv_state["gt4"]
                    conv_state["gt4"] += 1
                    ps = ps_mm.tile([C, QG], F32, tag="mm", name=f"cvg_{g4}")
                    for k in range(4):
                        mc = 4 * g4 + k
                        nc.tensor.matmul(
                            ps[:, k * C:(k + 1) * C],
                            lhsT=xf16[:, mc * C:(mc + 1) * C], rhs=wg16,
                            start=True, stop=True,
                        )
                    with nc.allow_low_precision(reason="gT storage dtype"):
                        nc.vector.tensor_copy(
                            out=gT_s[:, 4 * g4:4 * g4 + 4, :], in_=ps)

                def ensure_convs(mc):
                    # stay a few chunks ahead of the attention stream; finish
                    # early so the mm psum pool frees up for the rb tiles
                    while conv_state["phi"] < min((mc + 12) // 4, N // QG):
                        emit_phi_conv()
                    while conv_state["gt4"] < min((mc + 8) // 4 + 2, MC // 4):
                        emit_gt_conv4()

                ensure_convs(-1)

                def emit_tail(st, nsplit=2):
                    # rb accumulators are already closed (last absorb had
                    # stop=True at mc 31) -> pure recip/mul/tanh/gate chain.
                    # nsplit=4 shortens the serial chain for the final tail.
                    qg = st["qg"]
                    q0 = qg * GW
                    w = GW // nsplit
                    yns = []
                    for h in range(nsplit):
                        sl = slice(h * w, (h + 1) * w)
                        rbi = tailp.tile([C, w], F32, name=f"rbi_{qg}_{h}",
                                         tag=f"rbi{nsplit}")
                        rb_src = st["rb"][h * 2 // nsplit]
                        if nsplit > 2:
                            off = (h % (nsplit // 2)) * w
                            rb_src = rb_src[:, off:off + w]
                        nc.vector.reciprocal_approx_fast(out=rbi, in_=rb_src)
                        yn = tailp.tile([C, w], F32, name=f"yn_{qg}_{h}",
                                        tag=f"yn{nsplit}")
                        nc.vector.tensor_mul(out=yn, in0=st["yt"][:, sl],
                                             in1=rbi)
                        yns.append(yn)
                        # sigmoid(wy+bo)*xq == (tanh((wy+bo)/2) + 1) * (xq/2);
                        # the 0.5 factor is folded into Wg2, bo_s holds
                        # 0.5*bo_eff. tanh shares the exp ACT table set.
                        tn = tailp.tile([C, w], F32, name=f"tn_{qg}_{h}",
                                        tag=f"tn{nsplit}")
                        nc.scalar.activation(
                            out=tn, in_=yn, func=AF.Tanh, bias=bo_s,
                            scale=1.0,
                        )
                        o = tailp.tile([C, w], F32, name=f"o_{qg}_{h}",
                                       tag=f"o{nsplit}")
                        nc.vector.scalar_tensor_tensor(
                            out=o, in0=tn, scalar=1.0,
                            in1=xqh_s[:, q0 + h * w:q0 + (h + 1) * w],
                            op0=mybir.AluOpType.add, op1=mybir.AluOpType.mult,
                        )
                        nc.sync.dma_start(
                            out=out[:, q0 + h * w:q0 + (h + 1) * w], in_=o)

                # ---- software-pipelined attention stream. Per (qg, mc) step
                # three stages; the sc matmuls of step i+1 are EMITTED before
                # the av matmuls of step i so the in-order PE queue never
                # stalls on the exp of step i. ----
                qstate = []
                for qg in range(NQG):
                    qstate.append({
                        "qg": qg,
                        "yt": ps_y.tile([C, GW], F32, name=f"yt_{qg}",
                                        tag="yt"),
                        "levels": [None] * TREE_CAP,
                        "rb": None,           # [h0, h1] psum accumulators
                        "rb_started": [False, False],
                        "rb_pending": [],     # sum tiles awaiting absorption
                        "n_rb": 0,
                        # tree tiles: full 2^cap groups + one leftover level
                        "total_rb": DIRECT_RB[qg]
                        + (MC - DIRECT_RB[qg]) // (1 << TREE_CAP)
                        + (1 if (MC - DIRECT_RB[qg]) % (1 << TREE_CAP) else 0),
                        "sc": {},
                        "et": {},
                    })

                def stage_sc(st, mc):
                    qg = st["qg"]
                    if qg == 0:
                        ensure_convs(mc)
                        if mc == 12:
                            # xqh is first needed by the qg0 tail; emitting it
                            # here keeps the DVE clear during startup
                            nc.vector.tensor_scalar_mul(
                                out=xqh_s, in0=xq16, scalar1=0.5)
                    sc = ps_sc.tile([C, GW], F32, name=f"sc_{qg}_{mc}",
                                    tag="sc")
                    st["sc"][mc] = sc
                    q0 = qg * GW
                    for h in range(2):
                        nc.tensor.matmul(
                            sc[:, h * QG:(h + 1) * QG],
                            lhsT=phi_s[:, mc * C:(mc + 1) * C],
                            rhs=th_s[:, q0 + h * QG:q0 + (h + 1) * QG],
                            start=True, stop=True,
                        )

                def stage_exp(st, mc):
                    qg = st["qg"]
                    sc = st["sc"].pop(mc)
                    et = epool.tile([C, GW], AV_DT, name=f"et_{qg}_{mc}",
                                    tag="et")
                    st["et"][mc] = et
                    with nc.allow_low_precision(reason="exp output dtype"):
                        if (qg, mc) in DVE_EXP:
                            # Schraudolph: bits(e^x) = int16(A*x + BEXP);
                            # max(.,0) clamps scores < -88 to +0.0 (the
                            # int16 would go negative -> NaN bit pattern)
                            nc.vector.tensor_scalar(
                                out=et.bitcast(I16), in0=sc, scalar1=BEXP,
                                scalar2=0.0, op0=mybir.AluOpType.add,
                                op1=mybir.AluOpType.max)
                        else:
                            nc.scalar.activation(out=et, in_=sc, func=AF.Exp,
                                                 scale=1.0 / A_EXP)

                def rb_absorb(st, t, last):
                    for h in range(2):
                        nc.tensor.matmul(
                            st["rb"][h], lhsT=ones_sum,
                            rhs=t[:, h * QG:(h + 1) * QG],
                            start=not st["rb_started"][h], stop=last,
                        )
                        st["rb_started"][h] = True
                    st["n_rb"] += 1

                def stage_post(st, mc):
                    qg = st["qg"]
                    if qg == 1 and mc == 0:
                        emit_tail(qstate[0])
                    et = st["et"].pop(mc)
                    q0 = qg * GW
                    for h in range(2):
                        nc.tensor.matmul(
                            st["yt"][:, h * QG:(h + 1) * QG],
                            lhsT=gT_s[:, mc, :],
                            rhs=et[:, h * QG:(h + 1) * QG],
                            start=(mc == 0), stop=(mc == MC - 1),
                        )
                    if mc >= MC - DIRECT_RB[qg]:
                        st["rb_pending"].append(et)
                    else:
                        cur = et
                        lvl = 0
                        levels = st["levels"]
                        while lvl < TREE_CAP and levels[lvl] is not None:
                            t = tpool.tile([C, GW], AV_DT,
                                           name=f"tree_{qg}_{mc}_{lvl}",
                                           tag="tree")
                            with nc.allow_low_precision(reason="bf16 tree"):
                                nc.vector.tensor_add(out=t, in0=levels[lvl],
                                                     in1=cur)
                            levels[lvl] = None
                            cur = t
                            lvl += 1
                        if lvl == TREE_CAP:
                            st["rb_pending"].append(cur)
                        else:
                            levels[lvl] = cur
                        if mc == MC - DIRECT_RB[qg] - 1:
                            # flush leftover tree levels into one tile chain
                            rem = [t for t in levels if t is not None]
                            st["levels"] = [None] * TREE_CAP
                            while len(rem) > 1:
                                t = tpool.tile([C, GW], AV_DT,
                                               name=f"treefl_{qg}_{len(rem)}",
                                               tag="tree")
                                with nc.allow_low_precision(reason="bf16 tree"):
                                    nc.vector.tensor_add(out=t, in0=rem[0],
                                                         in1=rem[1])
                                rem = [t] + rem[2:]
                            st["rb_pending"].extend(rem)
                    if mc >= RB_CLAIM[qg]:
                        if st["rb"] is None:
                            st["rb"] = [
                                ps_mm.tile([C, QG], F32, tag="mm",
                                           name=f"rb_{qg}_0"),
                                ps_mm.tile([C, QG], F32, tag="mm",
                                           name=f"rb_{qg}_1"),
                            ]
                        while st["rb_pending"]:
                            t = st["rb_pending"].pop(0)
                            rb_absorb(st, t, st["n_rb"] == st["total_rb"] - 1)
                    if mc == MC - 1:
                        assert st["n_rb"] == st["total_rb"], (
                            st["n_rb"], st["total_rb"])

                # skew: sc leads exp by 1 step and av/absorb by 2, so the
                # in-order PE queue never waits on an exp that is <2 chunks
                # old, and ACT jitter doesn't stall the PE.
                steps = [(qg, mc) for qg in range(NQG) for mc in range(MC)]
                nsteps = len(steps)
                for i in range(nsteps + 2):
                    if i < nsteps:
                        stage_sc(qstate[steps[i][0]], steps[i][1])
                    if 1 <= i + 1 and 0 <= i - 1 < nsteps:
                        stage_exp(qstate[steps[i - 1][0]], steps[i - 1][1])
                    if 0 <= i - 2 < nsteps:
                        stage_post(qstate[steps[i - 2][0]], steps[i - 2][1])
                emit_tail(qstate[1])

    nc.compile()
    return nc


def get_program():
    if "nc" not in _CACHE:
        _CACHE["nc"] = build_program()
    return _CACHE["nc"]


def _f16_pack(a):
    # [C, k] f16 -> [C, k/2] f32 bit-view (little-endian pair packing
    # matches the device-side .bitcast(F16))
    a = np.ascontiguousarray(a.astype(np.float16))
    return a.view(np.float32)


def make_in_maps(x, Wg, bg, Wth, bth, Wph, bph, Wo, bo):
    xr = np.ascontiguousarray(x.reshape(B, C, N), np.float32)
    bo_eff = (Wo.astype(np.float64) @ bg.astype(np.float64)
              + bo.astype(np.float64)).astype(np.float32)
    Wg2 = 0.5 * (Wo.astype(np.float64) @ Wg.astype(np.float64))
    wblock = np.concatenate([
        _f16_pack(Wg2.T),
        _f16_pack(A_EXP * Wth.T),
        _f16_pack(Wph.T),
        (A_EXP * bth).reshape(C, 1).astype(np.float32),
        bph.reshape(C, 1).astype(np.float32),
        (0.5 * bo_eff).reshape(C, 1),
    ], axis=1)
    in_maps = []
    for core in range(N_CORES):
        b, qh = divmod(core, 2)
        mine = xr[b][:, qh * NQ:(qh + 1) * NQ]
        other = xr[b][:, (1 - qh) * NQ:(2 - qh) * NQ]
        x16 = _f16_pack(np.concatenate([mine, other], axis=1))
        packed = np.concatenate([x16, wblock], axis=1)
        in_maps.append({"inp": np.ascontiguousarray(packed)})
    return in_maps


def run(trace=False, **inputs):
    nc = get_program()
    in_maps = make_in_maps(**inputs)
    res = run_bass_kernel_spmd(nc, in_maps, core_ids=list(range(N_CORES)),
                               trace=trace)
    full = np.empty((B, C, N), np.float32)
    for core in range(N_CORES):
        b, qh = divmod(core, 2)
        full[b][:, qh * NQ:(qh + 1) * NQ] = res.results[core]["out"]
    return full.reshape(B, C, H, W), res


def kernel(**inputs) -> np.ndarray:
    out, _ = run(trace=False, **inputs)
    return out


# revision 39
# speedup vs baseline: 1.0356x; 1.0021x over previous
"""TRN2 Bass kernel for the NonLocal (full N^2 attention) block.

Contract: kernel(**inputs) takes the FULL inputs (x:[4,128,64,64] plus 4x
(W:[128,128], b:[128])) and returns the full [4,128,64,64] output.

Sharding: 8 cores = 4 batches x 2 query-halves (2048 query rows each).
Each core receives the full x[b] (keys/values span all 4096 positions) and
its query slice; outputs are disjoint [128,2048] slices -> no collectives.

Per-core pipeline (v2):
  phi   = Wph @ X + bph            [C, 4096]   (f32r conv, f16 storage)
  theta = A*(Wth @ Xq + bth)       [C, 2048]   (A = Schraudolph scale, f16)
  gT    = X^T @ Wg2^T              [4096, C]   Wg2 = 0.5*Wo@Wg  (Wo FOLDED:
                                               per-query normalization
                                               commutes through the channel
                                               conv, so the output conv is
                                               pre-applied to g and the wy
                                               matmul disappears)
  per 1024-wide q-group, streaming over 32 key-chunks of 128:
    scT  = phi_chunk^T @ theta_q   [128, 1024] (= A*score)
    E    = exp(scT/A)              ACT op (scale=1/A), OR on flagged chunks
           bitcast_bf16(int16(scT + BEXP))     (Schraudolph exp on the DVE,
                                               ~3% rel err, offloads the ACT)
    yT  += gT_chunk^T @ E          [C, 1024]   (PSUM accumulation)
    sums: bf16 pairwise tree on DVE; for the LAST q-group the late chunks
          bypass the tree and are absorbed directly into a persistent
          PSUM "rb" accumulator by ones-matmuls spread through the stream
          (shrinks the serial tail)
  rb    = allones^T @ sum_tiles    [C, 1024]   (partition reduce + broadcast)
  out   = (tanh(yT * recip(rb) + 0.5*bo_eff) + 1) * (Xq/2)

A ~12-matmul dummy burst on memset data runs at t=0 so the PE HAM clock
gate warms (~3.4us of busy) during the input DMA instead of 20us in.

All per-core inputs are packed into ONE [128, 4483] DRAM tensor so a single
dma_start loads everything behind one semaphore.
"""

import sys

for _p in ("/opt/trn_rl_repo",):
    if _p not in sys.path:
        sys.path.insert(0, _p)

import numpy as np

import concourse.bass as bass
import concourse.bacc as bacc
import concourse.mybir as mybir
import concourse.tile as tile
from concourse.bass_utils import run_bass_kernel_spmd

F32 = mybir.dt.float32
F32R = mybir.dt.float32r
F16 = mybir.dt.float16
BF16 = mybir.dt.bfloat16
I16 = mybir.dt.int16
AF = mybir.ActivationFunctionType

B, C, H, W = 4, 128, 64, 64
N = H * W            # 4096 key/value positions
NQ = N // 2          # 2048 query rows per core
QG = 512             # PSUM bank / max fp32 moving dim
GW = 2 * QG          # q-group width (1024)
NQG = NQ // GW       # 2 q-groups
MC = 32              # key chunks of 128
N_CORES = 8

SC_DT = F16
AV_DT = BF16

# Schraudolph fake-exp constants (bf16): bits(e^x) ~= int16(A*x + BEXP).
# A is folded into theta host-side; BEXP tuned for round-to-nearest convert.
A_EXP = 128.0 / np.log(2.0)          # 184.6650
BEXP_C0 = -5.6                       # centering offset, calibrated on host
BEXP = 16256.0 + BEXP_C0

# chunks whose exp runs on the DVE (Schraudolph) instead of the ACT engine.
# (qg, mc) pairs; keep them non-adjacent so each pairs with an ACT neighbor.
DVE_EXP = frozenset(
    [(0, mc) for mc in (22, 26, 30)]
    + [(1, mc) for mc in (13, 21, 28, 30)]
)

WARM_MMS = 17        # dummy FD=512 matmuls at t=0 to warm the PE clock gate
                     # AND bridge the DMA-gated conv phase so HAM never
                     # re-throttles before the stream saturates
DIRECT_RB = {0: 2, 1: 4}   # trailing chunks absorbed straight into rb psum
TREE_CAP = 3         # bf16 sum tree depth (tiles cover 2^cap chunks)
RB_CLAIM = {0: 22, 1: 6}   # chunk at which each qg claims its rb psum tiles

# packed input column offsets, in f32 columns. x and the conv weights are
# packed as f16 PAIRS host-side (halves the DMA, kills the on-chip casts);
# each core's x[b] is PERMUTED so its own query half comes first.
OFF_X16 = 0              # [C, N/2] f32 cols = [C, N] f16 x
OFF_WG = OFF_X16 + N // 2    # f16 (0.5*Wo@Wg).T, 64 f32 cols
OFF_WTH = OFF_WG + C // 2    # f16 (A*Wth).T
OFF_WPH = OFF_WTH + C // 2   # f16 Wph.T
OFF_BTH = OFF_WPH + C // 2   # f32 A*bth
OFF_BPH = OFF_BTH + 1
OFF_BO = OFF_BPH + 1     # holds 0.5*bo_eff (for the tanh-based sigmoid)
NW = OFF_BO + 1          # 2243

_CACHE = {}


def build_program():
    nc = bacc.Bacc("TRN2", target_bir_lowering=False, debug=False,
                   num_devices=N_CORES)

    inp = nc.declare_dram_parameter("inp", [C, NW], F32, isOutput=False)
    out = nc.declare_dram_parameter("out", [C, NQ], F32, isOutput=True)

    with tile.TileContext(nc) as tc:
        with (
            tc.tile_pool(name="const", bufs=1) as const,
            tc.tile_pool(name="big", bufs=1) as big,
            tc.tile_pool(name="epool", bufs=6) as epool,
            tc.tile_pool(name="tpool", bufs=10) as tpool,
            tc.tile_pool(name="tailp", bufs=2) as tailp,
        ):
            inp_s = big.tile([C, NW], F32)
            # weights/biases first, then f16-x in 4 ascending key chunks
            nc.sync.dma_start(out=inp_s[:, OFF_WG:], in_=inp[:, OFF_WG:])
            for dk in range(4):
                sl = slice(dk * (N // 8), (dk + 1) * (N // 8))
                nc.sync.dma_start(out=inp_s[:, sl], in_=inp[:, sl])

            xf16 = inp_s[:, OFF_X16:OFF_X16 + N // 2].bitcast(F16)  # [C, N]
            xq16 = xf16[:, :NQ]
            wg16 = inp_s[:, OFF_WG:OFF_WG + C // 2].bitcast(F16)
            wth16 = inp_s[:, OFF_WTH:OFF_WTH + C // 2].bitcast(F16)
            wph16 = inp_s[:, OFF_WPH:OFF_WPH + C // 2].bitcast(F16)
            bth_s = inp_s[:, OFF_BTH:OFF_BTH + 1]
            bph_s = inp_s[:, OFF_BPH:OFF_BPH + 1]
            bo_s = inp_s[:, OFF_BO:OFF_BO + 1]

            # all-ones [C, C] used as lhsT for the partition-reduce+broadcast
            # matmuls and as src for the PE warm-up burst
            ones_sum = const.tile([C, C], AV_DT)
            nc.vector.memset(ones_sum, 1.0)
            warm_src = const.tile([C, QG], AV_DT)
            nc.vector.memset(warm_src, 0.0)

            # preload the exp ACT table set while input DMA streams
            warm = const.tile([1, 1], F32)
            nc.scalar.activation(out=warm, in_=ones_sum[0:1, 0:1], func=AF.Exp)

            th_s = big.tile([C, NQ], SC_DT)
            # 0.5*xq, for out = (tanh(...) + 1) * (xq/2)
            xqh_s = big.tile([C, NQ], F32)
            phi_s = big.tile([C, N], SC_DT)
            gT_s = big.tile([C, MC, C], AV_DT)

            # ---- PSUM pools: sc 4 banks + yt 2 + mm 2 = 8 ----
            with (
                tc.tile_pool(name="ps_sc", bufs=2, space="PSUM") as ps_sc,
                tc.tile_pool(name="ps_y", bufs=1, space="PSUM") as ps_y,
                tc.tile_pool(name="ps_mm", bufs=2, space="PSUM") as ps_mm,
            ):
                # ---- PE warm-up burst: dummy matmuls on memset data keep the
                # PE busy ~4us so the HAM clock-gate opens during input DMA.
                # Writes land in the yt pool buffer; the first real AV matmul
                # (start=True) clears them. ----
                warm_ps = ps_y.tile([C, GW], F32, name="warm", tag="yt")
                for wi in range(WARM_MMS):
                    nc.tensor.matmul(warm_ps[:, :QG], lhsT=ones_sum,
                                     rhs=warm_src, start=True, stop=True)

                # ---- convs (bias adds on DVE). Only theta and the first
                # phi/gT tiles are emitted upfront; the rest interleave into
                # group 0's chunk loop. ----
                for j in range(NQ // QG):
                    ps = ps_mm.tile([C, QG], F32, tag="mm", name=f"cvt_{j}")
                    nc.tensor.matmul(
                        ps, lhsT=wth16, rhs=xf16[:, j * QG:(j + 1) * QG],
                        start=True, stop=True,
                    )
                    with nc.allow_low_precision(reason="theta storage dtype"):
                        nc.vector.tensor_scalar_add(
                            out=th_s[:, j * QG:(j + 1) * QG], in0=ps,
                            scalar1=bth_s)
                conv_state = {"phi": 0, "gt4": 0}

                def emit_phi_conv():
                    j = conv_state["phi"]
                    conv_state["phi"] += 1
                    ps = ps_mm.tile([C, QG], F32, tag="mm", name=f"cvp_{j}")
                    nc.tensor.matmul(
                        ps, lhsT=wph16, rhs=xf16[:, j * QG:(j + 1) * QG],
                        start=True, stop=True,
                    )
                    with nc.allow_low_precision(reason="phi storage dtype"):
                        nc.vector.tensor_scalar_add(
                            out=phi_s[:, j * QG:(j + 1) * QG], in0=ps,
                            scalar1=bph_s)

                def emit_gt_conv4():
                    g4 = conv_state["gt4"]
                    conv_state["gt4"] += 1
                    ps = ps_mm.tile([C, QG], F32, tag="mm", name=f"cvg_{g4}")
                    for k in range(4):
                        mc = 4 * g4 + k
                        nc.tensor.matmul(
                            ps[:, k * C:(k + 1) * C],
                            lhsT=xf16[:, mc * C:(mc + 1) * C], rhs=wg16,
                            start=True, stop=True,
                        )
                    with nc.allow_low_precision(reason="gT storage dtype"):
                        nc.vector.tensor_copy(
                            out=gT_s[:, 4 * g4:4 * g4 + 4, :], in_=ps)

                def ensure_convs(mc):
                    # stay a few chunks ahead of the attention stream; finish
                    # early so the mm psum pool frees up for the rb tiles
                    while conv_state["phi"] < min((mc + 12) // 4, N // QG):
                        emit_phi_conv()
                    while conv_state["gt4"] < min((mc + 8) // 4 + 2, MC // 4):
                        emit_gt_conv4()

                ensure_convs(-1)

                def emit_tail(st, nsplit=2):
                    # rb accumulators are already closed (last absorb had
                    # stop=True at mc 31) -> pure recip/mul/tanh/gate chain.
                    # nsplit=4 shortens the serial chain for the final tail.
                    qg = st["qg"]
                    q0 = qg * GW
                    w = GW // nsplit
                    yns = []
                    for h in range(nsplit):
                        sl = slice(h * w, (h + 1) * w)
                        rbi = tailp.tile([C, w], F32, name=f"rbi_{qg}_{h}",
                                         tag=f"rbi{nsplit}")
                        rb_src = st["rb"][h * 2 // nsplit]
                        if nsplit > 2:
                            off = (h % (nsplit // 2)) * w
                            rb_src = rb_src[:, off:off + w]
                        nc.vector.reciprocal_approx_fast(out=rbi, in_=rb_src)
                        yn = tailp.tile([C, w], F32, name=f"yn_{qg}_{h}",
                                        tag=f"yn{nsplit}")
                        nc.vector.tensor_mul(out=yn, in0=st["yt"][:, sl],
                                             in1=rbi)
                        yns.append(yn)
                        # sigmoid(wy+bo)*xq == (tanh((wy+bo)/2) + 1) * (xq/2);
                        # the 0.5 factor is folded into Wg2, bo_s holds
                        # 0.5*bo_eff. tanh shares the exp ACT table set.
                        tn = tailp.tile([C, w], F32, name=f"tn_{qg}_{h}",
                                        tag=f"tn{nsplit}")
                        nc.scalar.activation(
                            out=tn, in_=yn, func=AF.Tanh, bias=bo_s,
                            scale=1.0,
                        )
                        o = tailp.tile([C, w], F32, name=f"o_{qg}_{h}",
                                       tag=f"o{nsplit}")
                        nc.vector.scalar_tensor_tensor(
                            out=o, in0=tn, scalar=1.0,
                            in1=xqh_s[:, q0 + h * w:q0 + (h + 1) * w],
                            op0=mybir.AluOpType.add, op1=mybir.AluOpType.mult,
                        )
                        nc.sync.dma_start(
                            out=out[:, q0 + h * w:q0 + (h + 1) * w], in_=o)

                # ---- software-pipelined attention stream. Per (qg, mc) step
                # three stages; the sc matmuls of step i+1 are EMITTED before
                # the av matmuls of step i so the in-order PE queue never
                # stalls on the exp of step i. ----
                qstate = []
                for qg in range(NQG):
                    qstate.append({
                        "qg": qg,
                        "yt": ps_y.tile([C, GW], F32, name=f"yt_{qg}",
                                        tag="yt"),
                        "levels": [None] * TREE_CAP,
                        "rb": None,           # [h0, h1] psum accumulators
                        "rb_started": [False, False],
                        "rb_pending": [],     # sum tiles awaiting absorption
                        "n_rb": 0,
                        # tree tiles: full 2^cap groups + one leftover level
                        "total_rb": DIRECT_RB[qg]
                        + (MC - DIRECT_RB[qg]) // (1 << TREE_CAP)
                        + (1 if (MC - DIRECT_RB[qg]) % (1 << TREE_CAP) else 0),
                        "sc": {},
                        "et": {},
                    })

                def stage_sc(st, mc):
                    qg = st["qg"]
                    if qg == 0:
                        ensure_convs(mc)
                        if mc == 12:
                            # xqh is first needed by the qg0 tail; emitting it
                            # here keeps the DVE clear during startup
                            nc.vector.tensor_scalar_mul(
                                out=xqh_s, in0=xq16, scalar1=0.5)
                    sc = ps_sc.tile([C, GW], F32, name=f"sc_{qg}_{mc}",
                                    tag="sc")
                    st["sc"][mc] = sc
                    q0 = qg * GW
                    for h in range(2):
                        nc.tensor.matmul(
                            sc[:, h * QG:(h + 1) * QG],
                            lhsT=phi_s[:, mc * C:(mc + 1) * C],
                            rhs=th_s[:, q0 + h * QG:q0 + (h + 1) * QG],
                            start=True, stop=True,
                        )

                def stage_exp(st, mc):
                    qg = st["qg"]
                    sc = st["sc"].pop(mc)
                    et = epool.tile([C, GW], AV_DT, name=f"et_{qg}_{mc}",
                                    tag="et")
                    st["et"][mc] = et
                    with nc.allow_low_precision(reason="exp output dtype"):
                        if (qg, mc) in DVE_EXP:
                            # Schraudolph: bits(e^x) = int16(A*x + BEXP);
                            # max(.,0) clamps scores < -88 to +0.0 (the
                            # int16 would go negative -> NaN bit pattern)
                            nc.vector.tensor_scalar(
                                out=et.bitcast(I16), in0=sc, scalar1=BEXP,
                                scalar2=0.0, op0=mybir.AluOpType.add,
                                op1=mybir.AluOpType.max)
                        else:
                            nc.scalar.activation(out=et, in_=sc, func=AF.Exp,
                                                 scale=1.0 / A_EXP)

                def rb_absorb(st, t, last):
                    for h in range(2):
                        nc.tensor.matmul(
                            st["rb"][h], lhsT=ones_sum,
                            rhs=t[:, h * QG:(h + 1) * QG],
                            start=not st["rb_started"][h], stop=last,
                        )
                        st["rb_started"][h] = True
                    st["n_rb"] += 1

                def stage_post(st, mc):
                    qg = st["qg"]
                    if qg == 1 and mc == 0:
                        emit_tail(qstate[0])
                    et = st["et"].pop(mc)
                    q0 = qg * GW
                    for h in range(2):
                        nc.tensor.matmul(
                            st["yt"][:, h * QG:(h + 1) * QG],
                            lhsT=gT_s[:, mc, :],
                            rhs=et[:, h * QG:(h + 1) * QG],
                            start=(mc == 0), stop=(mc == MC - 1),
                        )
                    if mc >= MC - DIRECT_RB[qg]:
                        st["rb_pending"].append(et)
                    else:
                        cur = et
                        lvl = 0
                        levels = st["levels"]
                        while lvl < TREE_CAP and levels[lvl] is not None:
                            t = tpool.tile([C, GW], AV_DT,
                                           name=f"tree_{qg}_{mc}_{lvl}",
                                           tag="tree")
                            with nc.allow_low_precision(reason="bf16 tree"):
                                nc.vector.tensor_add(out=t, in0=levels[lvl],
                                                     in1=cur)
                            levels[lvl] = None
                            cur = t
                            lvl += 1
                        if lvl == TREE_CAP:
                            st["rb_pending"].append(cur)
                        else:
                            levels[lvl] = cur
                        if mc == MC - DIRECT_RB[qg] - 1:
                            # flush leftover tree levels into one tile chain
                            rem = [t for t in levels if t is not None]
                            st["levels"] = [None] * TREE_CAP
                            while len(rem) > 1:
                                t = tpool.tile([C, GW], AV_DT,
                                               name=f"treefl_{qg}_{len(rem)}",
                                               tag="tree")
                                with nc.allow_low_precision(reason="bf16 tree"):
                                    nc.vector.tensor_add(out=t, in0=rem[0],
                                                         in1=rem[1])
                                rem = [t] + rem[2:]
                            st["rb_pending"].extend(rem)
                    if mc >= RB_CLAIM[qg]:
                        if st["rb"] is None:
                            st["rb"] = [
                                ps_mm.tile([C, QG], F32, tag="mm",
                                           name=f"rb_{qg}_0"),
                                ps_mm.tile([C, QG], F32, tag="mm",
                                           name=f"rb_{qg}_1"),
                            ]
                        while st["rb_pending"]:
                            t = st["rb_pending"].pop(0)
                            rb_absorb(st, t, st["n_rb"] == st["total_rb"] - 1)
                    if mc == MC - 1:
                        assert st["n_rb"] == st["total_rb"], (
                            st["n_rb"], st["total_rb"])

                # skew: sc leads exp by 1 step and av/absorb by 2, so the
                # in-order PE queue never waits on an exp that is <2 chunks
                # old, and ACT jitter doesn't stall the PE.
                steps = [(qg, mc) for qg in range(NQG) for mc in range(MC)]
                nsteps = len(steps)
                for i in range(nsteps + 2):
                    if i < nsteps:
                        stage_sc(qstate[steps[i][0]], steps[i][1])
                    if 1 <= i + 1 and 0 <= i - 1 < nsteps:
                        stage_exp(qstate[steps[i - 1][0]], steps[i - 1][1])
                    if 0 <= i - 2 < nsteps:
                        stage_post(qstate[steps[i - 2][0]], steps[i - 2][1])
                emit_tail(qstate[1])

    nc.compile()
    return nc


def get_program():
    if "nc" not in _CACHE:
        _CACHE["nc"] = build_program()
    return _CACHE["nc"]


def _f16_pack(a):
    # [C, k] f16 -> [C, k/2] f32 bit-view (little-endian pair packing
    # matches the device-side .bitcast(F16))
    a = np.ascontiguousarray(a.astype(np.float16))
    return a.view(np.float32)


def make_in_maps(x, Wg, bg, Wth, bth, Wph, bph, Wo, bo):
    xr = np.ascontiguousarray(x.reshape(B, C, N), np.float32)
    bo_eff = (Wo.astype(np.float64) @ bg.astype(np.float64)
              + bo.astype(np.float64)).astype(np.float32)
    Wg2 = 0.5 * (Wo.astype(np.float64) @ Wg.astype(np.float64))
    wblock = np.concatenate([
        _f16_pack(Wg2.T),
        _f16_pack(A_EXP * Wth.T),
        _f16_pack(Wph.T),
        (A_EXP * bth).reshape(C, 1).astype(np.float32),
        bph.reshape(C, 1).astype(np.float32),
        (0.5 * bo_eff).reshape(C, 1),
    ], axis=1)
    in_maps = []
    for core in range(N_CORES):
        b, qh = divmod(core, 2)
        mine = xr[b][:, qh * NQ:(qh + 1) * NQ]
        other = xr[b][:, (1 - qh) * NQ:(2 - qh) * NQ]
        x16 = _f16_pack(np.concatenate([mine, other], axis=1))
        packed = np.concatenate([x16, wblock], axis=1)
        in_maps.append({"inp": np.ascontiguousarray(packed)})
    return in_maps


def run(trace=False, **inputs):
    nc = get_program()
    in_maps = make_in_maps(**inputs)
    res = run_bass_kernel_spmd(nc, in_maps, core_ids=list(range(N_CORES)),
                               trace=trace)
    full = np.empty((B, C, N), np.float32)
    for core in range(N_CORES):
        b, qh = divmod(core, 2)
        full[b][:, qh * NQ:(qh + 1) * NQ] = res.results[core]["out"]
    return full.reshape(B, C, H, W), res


def kernel(**inputs) -> np.ndarray:
    out, _ = run(trace=False, **inputs)
    return out
